# revision 49
# baseline (speedup 1.0000x reference)
"""Causal dense attention (Luong dot-product, key=value) on 8 Trainium2 cores.

Problem: B=4, Tq=Tv=4096, D=64, fp32.
  scores = Q @ V^T  (causal-masked, v_mask-masked), W = softmax(scores),
  out = (W @ V) * q_mask.

Strategy (v2)
-------------
144 (batch, q-chunk of 512, v-block of 512) blocks with vb <= qc (causal).
Per core: two "streams" of 9 blocks (stream A = one batch, stream B = its
sibling batch), aligned so pair p = (A[p], B[p]). Each stream has a fixed
canonical segment structure [4, 2, 2, 1] where every segment's blocks share
one (b, q-chunk) output group; the PV matmuls of a segment accumulate into
ONE [65, 512] psum tile (row 64 = softmax denominator via an appended
ones-column on V). Diagonal (vb == qc) blocks sit at pairs 5 and 8 in every
stream -> the device program is identical across cores (SPMD).

Per pair the device computes, in transposed layouts (S^T[v, q]):
    S^T = K_tile^T @ Q^T            (TensorE fp16, 2 blocks row-tiled, K=64)
    U   = exp(S^T)                  (split between ScalarE and VectorE)
    O^T[65, 512] += V_aug^T @ U     (TensorE bf16, accumulated per segment)

exp split: ScalarE runs the exact table-based Exp; VectorE computes a
bit-trick exp for a fixed subset of tiles: bf16 bits of exp(s) are
round_i16(s * 128/ln2 + (16256 - sigma)) -- one TENSOR_SCALAR per tile
(fp32 psum -> int16 sbuf, saturating round; bits reinterpreted as bf16).
That keeps ScalarE, VectorE and TensorE all under the matmul stream time.

Diagonal pairs mask the upper triangle by adding -1e9 into psum (VectorE)
before a ScalarE exp. v_mask is folded into V_aug on the host; q_mask is
applied after gather. Host sums per-(b, qc) partials (several segments and
cores) and divides by the denominator row.

A HAM warmup (~5 junk matmuls while the first input DMA flies) pushes the
PE's activity-gated clock (1.2 -> 2.4 GHz after one full ~3.4us busy
window) so real matmuls run warm. Input DMAs are dispatched from the Sync
queue (pair 0 split so the first matmul starts early), the triangular mask
from the Activation queue, and output DMAs from GpSimd's software queue --
three independent dispatchers.

This walrus encodes sync waits inline (one slot per 64B instruction), so a
BIR post-pass splits multi-wait instructions into standalone EventSemaphore
waits and elides same-engine self-waits (see install_bir_fixup).
"""
import math
import os
import numpy as np
import orjson

import concourse.bass as bass
import concourse.mybir as mybir
import concourse.tile as tile
from concourse.alu_op_type import AluOpType
from concourse.bass_utils import run_bass_kernel_spmd

F32 = mybir.dt.float32
F16 = mybir.dt.float16
BF16 = mybir.dt.bfloat16
I16 = mybir.dt.int16
EXP = mybir.ActivationFunctionType.Exp

B, T, D = 4, 4096, 64
NPAIR = 9
NEG = 1e9

ALPHA = 128.0 / math.log(2.0)      # bf16-bits exp slope
SIGMA = 3.0
BETA = 16256.0 - SIGMA             # bf16-bits exp offset (127 * 128 - sigma)

N_WARM = 6

TRACE = bool(int(os.environ.get("KERNEL_TRACE", "0")))
LAST_RESULTS = None  # BassKernelResults of the most recent run (for test.py)

# --------------------------------------------------------------- schedule
# Canonical per-batch streams: 4 streams x 4 segments of (qc, [vb...]).
# Segment sizes [4, 2, 2, 1]; diag blocks exactly at pair 5 (seg1[1]) and
# pair 8 (seg3[0]) in every stream.
STREAMS = [
    [(7, [0, 1, 2, 3]), (7, [6, 7]), (7, [4, 5]), (6, [6])],
    [(6, [0, 1, 2, 3]), (5, [4, 5]), (6, [4, 5]), (4, [4])],
    [(5, [0, 1, 2, 3]), (3, [2, 3]), (3, [0, 1]), (2, [2])],
    [(4, [0, 1, 2, 3]), (1, [0, 1]), (2, [0, 1]), (0, [0])],
]
# pair index -> (segment, index-within-segment)
PAIR_POS = [(0, 0), (0, 1), (0, 2), (0, 3), (1, 0), (1, 1), (2, 0), (2, 1),
            (3, 0)]
SEG_FIRST = {0, 4, 6, 8}     # pairs that open a new segment
SEG_LAST = {3: 0, 5: 1, 7: 2, 8: 3}   # pair -> segment it closes
DIAG_PAIRS = (5, 8)


def stream_blocks(kk):
    """Pair-ordered [(qc, vb)] * 9 for canonical stream kk."""
    out = []
    for qc, vbs in STREAMS[kk]:
        out += [(qc, v) for v in vbs]
    return out


# ---------------------------------------------------------------- BIR fixup
_SELF_ELIDE_ENGINES = ("PE", "Activation", "DVE")


def _split_multiwaits(raw: bytes) -> bytes:
    """Two rewrites on the serialized BIR:
    1. split multi-wait instructions into standalone EventSemaphore waits
       (this walrus encodes at most one inline wait per instruction);
    2. drop standalone same-engine self-waits (engine E waiting on E's own
       completion semaphore): engines execute and complete in order, so the
       threshold is satisfied by program order; increments are kept.
    """
    d = orjson.loads(raw)
    n = 0
    changed = False
    for fn in d.get("functions", []):
        for bb in fn.get("blocks", []):
            out = []
            for inst in bb.get("instructions", []):
                si = inst.get("sync_info")
                ow = (si or {}).get("on_wait") or []
                upd = (si or {}).get("on_update") or []
                eng = inst.get("engine")
                if (
                    inst.get("opcode") == "EventSemaphore"
                    and not upd
                    and eng in _SELF_ELIDE_ENGINES
                    and ow
                    and all(w["ant_name"].startswith(eng + "_") for w in ow)
                ):
                    changed = True
                    continue
                if len(ow) > 1:
                    changed = True
                    for w in ow[:-1]:
                        n += 1
                        out.append({
                            "debug": inst.get("debug"),
                            "engine": inst["engine"],
                            "ins": [],
                            "name": f"splitwait-{n}-{inst['name']}",
                            "opcode": "EventSemaphore",
                            "outs": [],
                            "sync_info": {"on_update": [], "on_wait": [w]},
                        })
                    si["on_wait"] = [ow[-1]]
                out.append(inst)
            bb["instructions"] = out
    return orjson.dumps(d) if changed else raw


def install_bir_fixup():
    import concourse.bass2jax as bass2jax
    orig = bass2jax._decompress_ant_bir
    if getattr(orig, "_is_splitwait_wrapper", False):
        return
    def patched(v):
        return _split_multiwaits(orig(v))
    patched._is_splitwait_wrapper = True
    bass2jax._decompress_ant_bir = patched


def install_ntff_hook():
    """Provide the missing antenv.axon_hooks glue so trace=True can capture
    NTFF profiles via the axon .so (used by test.py only)."""
    import sys
    import types
    try:
        import antenv.axon_hooks  # noqa: F401
        return
    except ImportError:
        pass
    import antenv
    mod = types.ModuleType("antenv.axon_hooks")
    _h = {}
    mod.set_axon_ntff_profile_hook = lambda h: _h.__setitem__("v", h)
    mod.get_axon_ntff_profile_hook = lambda: _h.get("v")
    sys.modules["antenv.axon_hooks"] = mod
    antenv.axon_hooks = mod
    from trn_agent_boot.trn_boot import _ntff_profile_via_ctypes
    mod.set_axon_ntff_profile_hook(
        _ntff_profile_via_ctypes("/opt/axon/libaxon_pjrt.so")
    )
    import concourse.bass_utils as bu
    bu.upload_artifacts = lambda tmpdir: f"file://{tmpdir}"


# ------------------------------------------------------------ device program
def build_program():
    nc = bass.Bass()
    in_d = nc.declare_dram_parameter("inb", [NPAIR, 128, 1544], F16,
                                     isOutput=False)
    # 0/1 lower-triangular (k <= q) mask for zeroing exp'd weights in SBUF
    trm_d = nc.declare_dram_parameter("trm", [128, 128], BF16, isOutput=False)
    # STT mask+bias plane: cols 0:128 = BETA + tri(-1e9), cols 128:512 = BETA
    stm_d = nc.declare_dram_parameter("stm", [128, 512], F32, isOutput=False)
    out_d = nc.declare_dram_parameter("out", [8, 65, 512], F32, isOutput=True)

    with tile.TileContext(nc) as tc:
        with (
            tc.tile_pool(name="sbin", bufs=5) as sbin,
            tc.tile_pool(name="upool", bufs=8) as upool,
            tc.tile_pool(name="single", bufs=1) as single,
            tc.tile_pool(name="ostage", bufs=4) as ostage,
            tc.tile_pool(name="psS", bufs=3, space="PSUM") as psS,
            tc.tile_pool(name="psO", bufs=2, space="PSUM") as psO,
        ):
            trm_t = single.tile([128, 128], BF16)
            stm_t = single.tile([128, 512], F32)
            warm = single.tile([128, 512], F16)
            wact = single.tile([128, 64], BF16)

            # pair-0 input: QK columns for jj=0 land first (gate the first
            # matmul); later K columns ride the Activation HWDGE queue in
            # parallel, va follows on Sync.
            it0 = sbin.tile([128, 1544], F16)
            nc.sync.dma_start(it0[:, 0:640], in_d[0][:, 0:640])
            nc.scalar.dma_start(it0[:, 640:1024], in_d[0][:, 640:1024])
            nc.sync.dma_start(it0[:, 1024:1544], in_d[0][:, 1024:1544])
            # masks via the Activation HWDGE queue (needed from pair 5 on)
            nc.scalar.dma_start(trm_t[:], trm_d[:])
            nc.scalar.dma_start(stm_t[:], stm_d[:])

            # HAM warmup: junk matmuls while the first input DMA flies, so
            # the PE's activity-gated clock ramps before real work; also
            # preload the exp spline tables (~1.3us) on ScalarE.
            nc.vector.memset(warm[:], 0.0)
            nc.scalar.activation(wact[:], warm[:, 0:64], EXP)
            psw = psS.tile([128, 512], F32, tag="psA", bufs=3)
            for _ in range(N_WARM):
                nc.tensor.matmul(psw[:], warm[:, 0:128], warm[:],
                                 start=True, stop=True)

            pending_pv = None
            pending_out = None     # (seg, oA, oB): copy+DMA after the
                                   # deferred PVs land in the next pair
            cur = {}               # stream ('A'|'B') -> active psO tile

            def emit_out(seg, oA, oB, act_only=False):
                # copies into one staging tile + a single 3D-AP DMA covering
                # both output slots, issued right as the segment closes so
                # the psO banks free before the next segment's first PV.
                # act_only routes both copies to ScalarE (diagonal pairs keep
                # VectorE busy with mask work).
                st = ostage.tile([65, 1024], F32, name=f"st{seg}")
                nc.scalar.copy(st[:, 0:512], oA[:])
                if act_only:
                    nc.scalar.copy(st[:, 512:1024], oB[:])
                else:
                    nc.vector.tensor_copy(st[:, 512:1024], oB[:])
                sta = st[:]
                src3 = bass.AP(tensor=sta.tensor, offset=sta.offset,
                               ap=[sta.ap[0], [512, 2], [1, 512]])
                od = out_d[2 * seg]
                dst3 = bass.AP(tensor=od.tensor, offset=od.offset,
                               ap=[od.ap[0], [65 * 512, 2], [1, 512]])
                nc.sync.dma_start(dst3, src3)

            for p in range(NPAIR):
                if p == 0:
                    it = it0
                else:
                    it = sbin.tile([128, 1544], F16)
                    nc.sync.dma_start(it[:], in_d[p])
                qt = it[:, 0:512]
                kt = it[:, 512:1024]
                va = it[:, 1024:1544].bitcast(BF16)
                diag = p in DIAG_PAIRS
                first = p in SEG_FIRST
                if first:
                    cur['A'] = psO.tile([65, 512], F32, tag="o", name=f"oA{p}")
                    cur['B'] = psO.tile([65, 512], F32, tag="o", name=f"oB{p}")
                oA, oB = cur['A'], cur['B']
                last_seg = SEG_LAST.get(p)

                def emit_qk(jj, q0):
                    # per-slot S tiles: each releases as soon as its own
                    # exp engine is done, decoupling the QK ring from the
                    # slower engine in any given jj step
                    psA = psS.tile([128, 512], F32, tag="psA", bufs=3,
                                   name=f"psA{p}_{jj}")
                    psB = psS.tile([128, 512], F32, tag="psB", bufs=3,
                                   name=f"psB{p}_{jj}")
                    nc.tensor.matmul(psA[:, q0:512],
                                     kt[0:64, jj * 128:(jj + 1) * 128],
                                     qt[0:64, q0:512], start=True, stop=True)
                    nc.tensor.matmul(psB[:, q0:512],
                                     kt[64:128, jj * 128:(jj + 1) * 128],
                                     qt[64:128, q0:512], start=True,
                                     stop=True)
                    return (psA, psB)

                def emit_act_pv(jj, q0, ps, defer_pv=False):
                    nonlocal pending_pv
                    psA, psB = ps
                    u = upool.tile([128, 1024], BF16)
                    # slot whose exp runs on ScalarE alternates by jj so
                    # both batches see the same error mix and PV waits
                    # only ~600ns for its half
                    psACT, psDVE = (psA, psB) if jj % 2 == 0 else (psB, psA)
                    offa = 0 if jj % 2 == 0 else 512
                    offv = 512 - offa
                    if diag:
                        # ScalarE slot: exact exp on raw (unmasked) psum,
                        # then the diagonal 128-col subblock of u is zeroed
                        # in SBUF by a tiny bf16 2x-mode VectorE multiply
                        # (exp(s)*0 == the reference's exp(s-1e9) == 0).
                        # VectorE slot: STT bit-trick with the mask+bias
                        # plane fused in (masked entries saturate the i16
                        # convert to 0x8000 = -0.0 bf16 -> zero weight).
                        w = 512 - q0
                        nc.scalar.activation(u[:, offa + q0:offa + 512],
                                             psACT[:, q0:512], EXP)
                        ud = u[:, offa + q0:offa + q0 + 128]
                        nc.vector.tensor_mul(ud, ud, trm_t[:])
                        nc.vector.scalar_tensor_tensor(
                            u[:, offv + q0:offv + 512].bitcast(I16),
                            psDVE[:, q0:512], ALPHA, stm_t[:, 0:w],
                            AluOpType.mult, AluOpType.add)
                    else:
                        # slot-split exp: ScalarE exact exp on one slot,
                        # VectorE bit-trick (bf16 bits = sat_round_i16(
                        # s * ALPHA + BETA)) on the other
                        nc.scalar.activation(u[:, offa:offa + 512],
                                             psACT[:], EXP)
                        nc.vector.tensor_scalar(
                            u[:, offv:offv + 512].bitcast(I16),
                            psDVE[:], ALPHA, BETA,
                            AluOpType.mult, AluOpType.add)

                    start = first and jj == 0
                    stop = last_seg is not None and jj == 3

                    def pv(oA=oA, oB=oB, va=va, u=u, jj=jj, q0=q0,
                           start=start, stop=stop):
                        nc.tensor.matmul(oA[:, q0:512],
                                         va[:, jj * 65:(jj + 1) * 65],
                                         u[:, q0:512],
                                         start=start, stop=stop,
                                         skip_group_check=True)
                        nc.tensor.matmul(oB[:, q0:512],
                                         va[:, 260 + jj * 65:260 + (jj + 1) * 65],
                                         u[:, 512 + q0:1024],
                                         start=start, stop=stop,
                                         skip_group_check=True)
                    if defer_pv:
                        pending_pv = pv
                    else:
                        pv()

                final = p == NPAIR - 1

                def emit_final_chunk(c0, c1):
                    # pair 8 is a single diagonal block: PV(jj) only touches
                    # columns >= jj*128, so cols [0:c) settle progressively;
                    # ship chunks (both slots, one 3D-AP DMA each) as they
                    # finalize to shorten the tail. Copies on ScalarE;
                    # VectorE carries the mask work.
                    w = c1 - c0
                    st = ostage.tile([65, 512], F32, name=f"fc{c0}")
                    nc.scalar.copy(st[:, 0:w], oA[:, c0:c1])
                    nc.scalar.copy(st[:, w:2 * w], oB[:, c0:c1])
                    sta = st[:]
                    src3 = bass.AP(tensor=sta.tensor, offset=sta.offset,
                                   ap=[sta.ap[0], [w, 2], [1, w]])
                    od = out_d[2 * last_seg]
                    dst3 = bass.AP(tensor=od.tensor,
                                   offset=od.offset + c0,
                                   ap=[od.ap[0], [65 * 512, 2], [1, w]])
                    nc.sync.dma_start(dst3, src3)

                for jj in range(4):
                    q0 = jj * 128 if diag else 0
                    ps = emit_qk(jj, q0)
                    if jj == 0:
                        if pending_pv is not None:
                            # previous pair's last PV issues after our first
                            # QK so PE's in-order queue doesn't stall it
                            pending_pv()
                            pending_pv = None
                        if pending_out is not None:
                            # previous segment's copies right behind its
                            # deferred PVs: psO frees ~1.2us before this
                            # pair's first PV needs the banks
                            emit_out(*pending_out)
                            pending_out = None
                    emit_act_pv(jj, q0, ps, defer_pv=(jj == 3 and not final))
                    if final and jj == 1:
                        emit_final_chunk(0, 256)
                    elif final and jj == 2:
                        emit_final_chunk(256, 384)
                    elif final and jj == 3:
                        emit_final_chunk(384, 512)
                if last_seg is not None and not final:
                    pending_out = (last_seg, oA, oB)
    return nc


_NC_CACHE = None


def _get_nc():
    global _NC_CACHE
    if _NC_CACHE is None:
        _NC_CACHE = build_program()
    return _NC_CACHE


# -------------------------------------------------------------- host wrapper
def kernel(query, value, q_mask, v_mask):
    install_bir_fixup()
    if TRACE:
        install_ntff_hook()
    global LAST_RESULTS

    query = np.asarray(query, dtype=np.float32)
    value = np.asarray(value, dtype=np.float32)
    q_mask = np.asarray(q_mask).astype(bool)
    v_mask = np.asarray(v_mask).astype(bool)

    # v_mask folded into the PV stationary operand: V_aug = [V * m | m].
    # A masked key then contributes exp(s)*0 to both numerator and
    # denominator -- exactly the reference's exp(s - 1e9) == 0 in fp32.
    import ml_dtypes
    bf16 = ml_dtypes.bfloat16
    vm = v_mask.astype(np.float32)
    v_aug = np.concatenate([value * vm[:, :, None], vm[:, :, None]], axis=2)
    v_aug = v_aug.astype(bf16)                              # [B, T, 65]
    q_t = np.ascontiguousarray(np.swapaxes(query, 1, 2)).astype(np.float16)
    k_t = np.ascontiguousarray(np.swapaxes(value, 1, 2)).astype(np.float16)

    tri_neg = np.where(np.tril(np.ones((128, 128), dtype=bool), -1), -NEG,
                       0.0).astype(np.float32)
    # STT plane: (ps*ALPHA) + stm; masked entries drive the i16 convert into
    # saturation (-32768 = -0.0 bf16), unmasked get the BETA offset
    stm = np.full((128, 512), BETA, dtype=np.float32)
    stm[:, 0:128] += tri_neg
    # 0/1 keep-mask (key kk attends query col c iff kk <= c) for zeroing
    # exp'd weights of the ScalarE slot's diagonal subblock
    trm = np.triu(np.ones((128, 128), dtype=np.float32)).astype(bf16)

    in_maps = []
    for c in range(8):
        bp, kk = divmod(c, 4)
        blocks = stream_blocks(kk)
        inb = np.empty((NPAIR, 128, 1544), dtype=np.float16)
        for p in range(NPAIR):
            qc, vb = blocks[p]
            for s, b in enumerate((2 * bp, 2 * bp + 1)):
                rows = slice(64 * s, 64 * s + 64)
                inb[p, rows, 0:512] = q_t[b, :, qc * 512:(qc + 1) * 512]
                inb[p, rows, 512:1024] = k_t[b, :, vb * 512:(vb + 1) * 512]
                # va: bf16 bytes viewed as fp16; col 1024 + 260*s + 65*jj + e,
                # row r -> V_aug[b, vb*512 + jj*128 + r, e]
                blk = v_aug[b, vb * 512:(vb + 1) * 512, :].reshape(4, 128, 65)
                inb[p, :, 1024 + 260 * s:1024 + 260 * (s + 1)] = (
                    blk.transpose(1, 0, 2).reshape(128, 260).view(np.float16)
                )
        in_maps.append({"inb": inb, "trm": trm, "stm": stm})

    nc = _get_nc()
    res = run_bass_kernel_spmd(
        nc, in_maps, list(range(8)),
        trace=TRACE,
        trace_cores=list(range(8)) if TRACE else None,
    )
    LAST_RESULTS = res

    # gather: sum per-(b, qc) segment partials, normalize, transpose back
    acc = np.zeros((B, 8, 65, 512), dtype=np.float64)
    for c in range(8):
        bp, kk = divmod(c, 4)
        o = res.results[c]["out"]  # [8, 65, 512]
        for seg in range(4):
            qc = STREAMS[kk][seg][0]
            for s, b in enumerate((2 * bp, 2 * bp + 1)):
                acc[b, qc] += o[2 * seg + s]
    denom = acc[:, :, 64:65, :]
    denom = np.where(denom == 0.0, 1.0, denom)
    o_t = acc[:, :, 0:64, :] / denom                      # [B, 8, 64, 512]
    out = o_t.transpose(0, 1, 3, 2).reshape(B, T, D)      # [B, T, D]
    out = out * q_mask[:, :, None]
    return out.astype(np.float32)


# revision 50
# speedup vs baseline: 1.0317x; 1.0317x over previous
"""Causal dense attention (Luong dot-product, key=value) on 8 Trainium2 cores.

Problem: B=4, Tq=Tv=4096, D=64, fp32.
  scores = Q @ V^T  (causal-masked, v_mask-masked), W = softmax(scores),
  out = (W @ V) * q_mask.

Strategy (v2)
-------------
144 (batch, q-chunk of 512, v-block of 512) blocks with vb <= qc (causal).
Per core: two "streams" of 9 blocks (stream A = one batch, stream B = its
sibling batch), aligned so pair p = (A[p], B[p]). Each stream has a fixed
canonical segment structure [4, 2, 2, 1] where every segment's blocks share
one (b, q-chunk) output group; the PV matmuls of a segment accumulate into
ONE [65, 512] psum tile (row 64 = softmax denominator via an appended
ones-column on V). Diagonal (vb == qc) blocks sit at pairs 5 and 8 in every
stream -> the device program is identical across cores (SPMD).

Per pair the device computes, in transposed layouts (S^T[v, q]):
    S^T = K_tile^T @ Q^T            (TensorE fp16, 2 blocks row-tiled, K=64)
    U   = exp(S^T)                  (split between ScalarE and VectorE)
    O^T[65, 512] += V_aug^T @ U     (TensorE bf16, accumulated per segment)

exp split: ScalarE runs the exact table-based Exp; VectorE computes a
bit-trick exp for a fixed subset of tiles: bf16 bits of exp(s) are
round_i16(s * 128/ln2 + (16256 - sigma)) -- one TENSOR_SCALAR per tile
(fp32 psum -> int16 sbuf, saturating round; bits reinterpreted as bf16).
That keeps ScalarE, VectorE and TensorE all under the matmul stream time.

Diagonal pairs mask the upper triangle by adding -1e9 into psum (VectorE)
before a ScalarE exp. v_mask is folded into V_aug on the host; q_mask is
applied after gather. Host sums per-(b, qc) partials (several segments and
cores) and divides by the denominator row.

A HAM warmup (~5 junk matmuls while the first input DMA flies) pushes the
PE's activity-gated clock (1.2 -> 2.4 GHz after one full ~3.4us busy
window) so real matmuls run warm. Input DMAs are dispatched from the Sync
queue (pair 0 split so the first matmul starts early), the triangular mask
from the Activation queue, and output DMAs from GpSimd's software queue --
three independent dispatchers.

This walrus encodes sync waits inline (one slot per 64B instruction), so a
BIR post-pass splits multi-wait instructions into standalone EventSemaphore
waits and elides same-engine self-waits (see install_bir_fixup).
"""
import math
import os
import numpy as np
import orjson

import concourse.bass as bass
import concourse.mybir as mybir
import concourse.tile as tile
from concourse.alu_op_type import AluOpType
from concourse.bass_utils import run_bass_kernel_spmd

F32 = mybir.dt.float32
F16 = mybir.dt.float16
BF16 = mybir.dt.bfloat16
I16 = mybir.dt.int16
EXP = mybir.ActivationFunctionType.Exp

B, T, D = 4, 4096, 64
NPAIR = 9
NEG = 1e9

ALPHA = 128.0 / math.log(2.0)      # bf16-bits exp slope
SIGMA = 3.0
BETA = 16256.0 - SIGMA             # bf16-bits exp offset (127 * 128 - sigma)

N_WARM = 6

TRACE = bool(int(os.environ.get("KERNEL_TRACE", "0")))
LAST_RESULTS = None  # BassKernelResults of the most recent run (for test.py)

# --------------------------------------------------------------- schedule
# Canonical per-batch streams: 4 streams x 4 segments of (qc, [vb...]).
# Segment sizes [4, 2, 2, 1]; diag blocks exactly at pair 5 (seg1[1]) and
# pair 8 (seg3[0]) in every stream.
STREAMS = [
    [(7, [0, 1, 2, 3]), (7, [6, 7]), (7, [4, 5]), (6, [6])],
    [(6, [0, 1, 2, 3]), (5, [4, 5]), (6, [4, 5]), (4, [4])],
    [(5, [0, 1, 2, 3]), (3, [2, 3]), (3, [0, 1]), (2, [2])],
    [(4, [0, 1, 2, 3]), (1, [0, 1]), (2, [0, 1]), (0, [0])],
]
# pair index -> (segment, index-within-segment)
PAIR_POS = [(0, 0), (0, 1), (0, 2), (0, 3), (1, 0), (1, 1), (2, 0), (2, 1),
            (3, 0)]
SEG_FIRST = {0, 4, 6, 8}     # pairs that open a new segment
SEG_LAST = {3: 0, 5: 1, 7: 2, 8: 3}   # pair -> segment it closes
DIAG_PAIRS = (5, 8)


def stream_blocks(kk):
    """Pair-ordered [(qc, vb)] * 9 for canonical stream kk."""
    out = []
    for qc, vbs in STREAMS[kk]:
        out += [(qc, v) for v in vbs]
    return out


# ---------------------------------------------------------------- BIR fixup
_SELF_ELIDE_ENGINES = ("PE", "Activation", "DVE")


def _split_multiwaits(raw: bytes) -> bytes:
    """Two rewrites on the serialized BIR:
    1. split multi-wait instructions into standalone EventSemaphore waits
       (this walrus encodes at most one inline wait per instruction);
    2. drop standalone same-engine self-waits (engine E waiting on E's own
       completion semaphore): engines execute and complete in order, so the
       threshold is satisfied by program order; increments are kept.
    """
    d = orjson.loads(raw)
    n = 0
    changed = False
    for fn in d.get("functions", []):
        for bb in fn.get("blocks", []):
            out = []
            for inst in bb.get("instructions", []):
                si = inst.get("sync_info")
                ow = (si or {}).get("on_wait") or []
                upd = (si or {}).get("on_update") or []
                eng = inst.get("engine")
                if (
                    inst.get("opcode") == "EventSemaphore"
                    and not upd
                    and eng in _SELF_ELIDE_ENGINES
                    and ow
                    and all(w["ant_name"].startswith(eng + "_") for w in ow)
                ):
                    changed = True
                    continue
                if len(ow) > 1:
                    changed = True
                    for w in ow[:-1]:
                        n += 1
                        out.append({
                            "debug": inst.get("debug"),
                            "engine": inst["engine"],
                            "ins": [],
                            "name": f"splitwait-{n}-{inst['name']}",
                            "opcode": "EventSemaphore",
                            "outs": [],
                            "sync_info": {"on_update": [], "on_wait": [w]},
                        })
                    si["on_wait"] = [ow[-1]]
                out.append(inst)
            bb["instructions"] = out
    return orjson.dumps(d) if changed else raw


def install_bir_fixup():
    import concourse.bass2jax as bass2jax
    orig = bass2jax._decompress_ant_bir
    if getattr(orig, "_is_splitwait_wrapper", False):
        return
    def patched(v):
        return _split_multiwaits(orig(v))
    patched._is_splitwait_wrapper = True
    bass2jax._decompress_ant_bir = patched


def install_ntff_hook():
    """Provide the missing antenv.axon_hooks glue so trace=True can capture
    NTFF profiles via the axon .so (used by test.py only)."""
    import sys
    import types
    try:
        import antenv.axon_hooks  # noqa: F401
        return
    except ImportError:
        pass
    import antenv
    mod = types.ModuleType("antenv.axon_hooks")
    _h = {}
    mod.set_axon_ntff_profile_hook = lambda h: _h.__setitem__("v", h)
    mod.get_axon_ntff_profile_hook = lambda: _h.get("v")
    sys.modules["antenv.axon_hooks"] = mod
    antenv.axon_hooks = mod
    from trn_agent_boot.trn_boot import _ntff_profile_via_ctypes
    mod.set_axon_ntff_profile_hook(
        _ntff_profile_via_ctypes("/opt/axon/libaxon_pjrt.so")
    )
    import concourse.bass_utils as bu
    bu.upload_artifacts = lambda tmpdir: f"file://{tmpdir}"


# ------------------------------------------------------------ device program
def build_program():
    nc = bass.Bass()
    in_d = nc.declare_dram_parameter("inb", [NPAIR, 128, 1544], F16,
                                     isOutput=False)
    # 0/1 lower-triangular (k <= q) mask for zeroing exp'd weights in SBUF
    trm_d = nc.declare_dram_parameter("trm", [128, 128], BF16, isOutput=False)
    # STT mask+bias plane: cols 0:128 = BETA + tri(-1e9), cols 128:512 = BETA
    stm_d = nc.declare_dram_parameter("stm", [128, 512], F32, isOutput=False)
    out_d = nc.declare_dram_parameter("out", [8, 65, 512], F32, isOutput=True)

    with tile.TileContext(nc) as tc:
        with (
            tc.tile_pool(name="sbin", bufs=5) as sbin,
            tc.tile_pool(name="upool", bufs=8) as upool,
            tc.tile_pool(name="single", bufs=1) as single,
            tc.tile_pool(name="ostage", bufs=4) as ostage,
            tc.tile_pool(name="psS", bufs=3, space="PSUM") as psS,
            tc.tile_pool(name="psO", bufs=2, space="PSUM") as psO,
        ):
            trm_t = single.tile([128, 128], BF16)
            stm_t = single.tile([128, 512], F32)
            warm = single.tile([128, 512], F16)
            wact = single.tile([128, 64], BF16)

            # pair-0 input: QK columns for jj=0 land first (gate the first
            # matmul); later K columns ride the Activation HWDGE queue in
            # parallel, va follows on Sync.
            it0 = sbin.tile([128, 1544], F16)
            nc.sync.dma_start(it0[:, 0:640], in_d[0][:, 0:640])
            nc.scalar.dma_start(it0[:, 640:1024], in_d[0][:, 640:1024])
            nc.sync.dma_start(it0[:, 1024:1544], in_d[0][:, 1024:1544])
            # masks via the Activation HWDGE queue (needed from pair 5 on)
            nc.scalar.dma_start(trm_t[:], trm_d[:])
            nc.scalar.dma_start(stm_t[:], stm_d[:])

            # HAM warmup: junk matmuls while the first input DMA flies, so
            # the PE's activity-gated clock ramps before real work; also
            # preload the exp spline tables (~1.3us) on ScalarE.
            nc.vector.memset(warm[:], 0.0)
            nc.scalar.activation(wact[:], warm[:, 0:64], EXP)
            psw = psS.tile([128, 1024], F32, tag="ps")
            for _ in range(N_WARM):
                nc.tensor.matmul(psw[:, 0:512], warm[:, 0:128], warm[:],
                                 start=True, stop=True)

            pending_pv = None
            cur = {}               # stream ('A'|'B') -> active psO tile

            def emit_out(seg, oA, oB, act_only=False):
                # copies into one staging tile + a single 3D-AP DMA covering
                # both output slots, issued right as the segment closes so
                # the psO banks free before the next segment's first PV.
                # act_only routes both copies to ScalarE (diagonal pairs keep
                # VectorE busy with mask work).
                st = ostage.tile([65, 1024], F32, name=f"st{seg}")
                nc.scalar.copy(st[:, 0:512], oA[:])
                if act_only:
                    nc.scalar.copy(st[:, 512:1024], oB[:])
                else:
                    nc.vector.tensor_copy(st[:, 512:1024], oB[:])
                sta = st[:]
                src3 = bass.AP(tensor=sta.tensor, offset=sta.offset,
                               ap=[sta.ap[0], [512, 2], [1, 512]])
                od = out_d[2 * seg]
                dst3 = bass.AP(tensor=od.tensor, offset=od.offset,
                               ap=[od.ap[0], [65 * 512, 2], [1, 512]])
                nc.sync.dma_start(dst3, src3)

            for p in range(NPAIR):
                if p == 0:
                    it = it0
                else:
                    it = sbin.tile([128, 1544], F16)
                    nc.sync.dma_start(it[:], in_d[p])
                qt = it[:, 0:512]
                kt = it[:, 512:1024]
                va = it[:, 1024:1544].bitcast(BF16)
                diag = p in DIAG_PAIRS
                first = p in SEG_FIRST
                if first:
                    cur['A'] = psO.tile([65, 512], F32, tag="o", name=f"oA{p}")
                    cur['B'] = psO.tile([65, 512], F32, tag="o", name=f"oB{p}")
                oA, oB = cur['A'], cur['B']
                last_seg = SEG_LAST.get(p)

                def emit_qk(jj, q0):
                    ps = psS.tile([128, 1024], F32, tag="ps")
                    nc.tensor.matmul(ps[:, q0:512],
                                     kt[0:64, jj * 128:(jj + 1) * 128],
                                     qt[0:64, q0:512], start=True, stop=True)
                    nc.tensor.matmul(ps[:, 512 + q0:1024],
                                     kt[64:128, jj * 128:(jj + 1) * 128],
                                     qt[64:128, q0:512], start=True,
                                     stop=True)
                    return ps

                def emit_act_pv(jj, q0, ps, defer_pv=False):
                    nonlocal pending_pv
                    u = upool.tile([128, 1024], BF16)
                    if diag:
                        # slot-split, alternating by jj. ScalarE slot: exact
                        # exp on raw (unmasked) psum, then the diagonal
                        # 128-col subblock of u is zeroed in SBUF by a tiny
                        # bf16 2x-mode VectorE multiply (exp(s)*0 == the
                        # reference's exp(s-1e9) == 0). VectorE slot: STT
                        # bit-trick with the mask+bias plane fused in
                        # (masked entries saturate the i16 convert to
                        # 0x8000 = -0.0 bf16 -> zero weight).
                        w = 512 - q0
                        offa = (jj % 2) * 512
                        offv = 512 - offa
                        nc.scalar.activation(u[:, offa + q0:offa + 512],
                                             ps[:, offa + q0:offa + 512],
                                             EXP)
                        ud = u[:, offa + q0:offa + q0 + 128]
                        nc.vector.tensor_mul(ud, ud, trm_t[:])
                        nc.vector.scalar_tensor_tensor(
                            u[:, offv + q0:offv + 512].bitcast(I16),
                            ps[:, offv + q0:offv + 512], ALPHA,
                            stm_t[:, 0:w],
                            AluOpType.mult, AluOpType.add)
                    else:
                        # slot-split exp: ScalarE runs exact exp on one
                        # 512-half, VectorE the bit-trick (bf16 bits =
                        # sat_round_i16(s * ALPHA + BETA)) on the other;
                        # alternate halves by jj so both batches see the
                        # same error mix and PV waits only ~600ns.
                        ha = slice(0, 512) if jj % 2 == 0 else slice(512, 1024)
                        hv = slice(512, 1024) if jj % 2 == 0 else slice(0, 512)
                        nc.scalar.activation(u[:, ha], ps[:, ha], EXP)
                        nc.vector.tensor_scalar(u[:, hv].bitcast(I16),
                                                ps[:, hv], ALPHA, BETA,
                                                AluOpType.mult, AluOpType.add)

                    start = first and jj == 0
                    stop = last_seg is not None and jj == 3

                    def pv(oA=oA, oB=oB, va=va, u=u, jj=jj, q0=q0,
                           start=start, stop=stop):
                        nc.tensor.matmul(oA[:, q0:512],
                                         va[:, jj * 65:(jj + 1) * 65],
                                         u[:, q0:512],
                                         start=start, stop=stop,
                                         skip_group_check=True)
                        nc.tensor.matmul(oB[:, q0:512],
                                         va[:, 260 + jj * 65:260 + (jj + 1) * 65],
                                         u[:, 512 + q0:1024],
                                         start=start, stop=stop,
                                         skip_group_check=True)
                    if defer_pv:
                        pending_pv = pv
                    else:
                        pv()

                final = p == NPAIR - 1

                def emit_final_chunk(c0, c1):
                    # pair 8 is a single diagonal block: PV(jj) only touches
                    # columns >= jj*128, so cols [0:c) settle progressively;
                    # ship chunks (both slots, one 3D-AP DMA each) as they
                    # finalize to shorten the tail. Copies on ScalarE;
                    # VectorE carries the mask work.
                    w = c1 - c0
                    st = ostage.tile([65, 512], F32, name=f"fc{c0}")
                    nc.scalar.copy(st[:, 0:w], oA[:, c0:c1])
                    nc.scalar.copy(st[:, w:2 * w], oB[:, c0:c1])
                    sta = st[:]
                    src3 = bass.AP(tensor=sta.tensor, offset=sta.offset,
                                   ap=[sta.ap[0], [w, 2], [1, w]])
                    od = out_d[2 * last_seg]
                    dst3 = bass.AP(tensor=od.tensor,
                                   offset=od.offset + c0,
                                   ap=[od.ap[0], [65 * 512, 2], [1, w]])
                    nc.sync.dma_start(dst3, src3)

                for jj in range(4):
                    q0 = jj * 128 if diag else 0
                    ps = emit_qk(jj, q0)
                    if jj == 0 and pending_pv is not None:
                        # previous pair's last PV issues after our first
                        # QK so PE's in-order queue doesn't stall it
                        pending_pv()
                        pending_pv = None
                    emit_act_pv(jj, q0, ps,
                                defer_pv=(jj == 3 and last_seg is None))
                    if final and jj == 1:
                        emit_final_chunk(0, 256)
                    elif final and jj == 2:
                        emit_final_chunk(256, 384)
                    elif final and jj == 3:
                        emit_final_chunk(384, 512)
                if last_seg is not None and not final:
                    emit_out(last_seg, oA, oB)
    return nc


_NC_CACHE = None


def _get_nc():
    global _NC_CACHE
    if _NC_CACHE is None:
        _NC_CACHE = build_program()
    return _NC_CACHE


# -------------------------------------------------------------- host wrapper
def kernel(query, value, q_mask, v_mask):
    install_bir_fixup()
    if TRACE:
        install_ntff_hook()
    global LAST_RESULTS

    query = np.asarray(query, dtype=np.float32)
    value = np.asarray(value, dtype=np.float32)
    q_mask = np.asarray(q_mask).astype(bool)
    v_mask = np.asarray(v_mask).astype(bool)

    # v_mask folded into the PV stationary operand: V_aug = [V * m | m].
    # A masked key then contributes exp(s)*0 to both numerator and
    # denominator -- exactly the reference's exp(s - 1e9) == 0 in fp32.
    import ml_dtypes
    bf16 = ml_dtypes.bfloat16
    vm = v_mask.astype(np.float32)
    v_aug = np.concatenate([value * vm[:, :, None], vm[:, :, None]], axis=2)
    v_aug = v_aug.astype(bf16)                              # [B, T, 65]
    q_t = np.ascontiguousarray(np.swapaxes(query, 1, 2)).astype(np.float16)
    k_t = np.ascontiguousarray(np.swapaxes(value, 1, 2)).astype(np.float16)

    tri_neg = np.where(np.tril(np.ones((128, 128), dtype=bool), -1), -NEG,
                       0.0).astype(np.float32)
    # STT plane: (ps*ALPHA) + stm; masked entries drive the i16 convert into
    # saturation (-32768 = -0.0 bf16), unmasked get the BETA offset
    stm = np.full((128, 512), BETA, dtype=np.float32)
    stm[:, 0:128] += tri_neg
    # 0/1 keep-mask (key kk attends query col c iff kk <= c) for zeroing
    # exp'd weights of the ScalarE slot's diagonal subblock
    trm = np.triu(np.ones((128, 128), dtype=np.float32)).astype(bf16)

    in_maps = []
    for c in range(8):
        bp, kk = divmod(c, 4)
        blocks = stream_blocks(kk)
        inb = np.empty((NPAIR, 128, 1544), dtype=np.float16)
        for p in range(NPAIR):
            qc, vb = blocks[p]
            for s, b in enumerate((2 * bp, 2 * bp + 1)):
                rows = slice(64 * s, 64 * s + 64)
                inb[p, rows, 0:512] = q_t[b, :, qc * 512:(qc + 1) * 512]
                inb[p, rows, 512:1024] = k_t[b, :, vb * 512:(vb + 1) * 512]
                # va: bf16 bytes viewed as fp16; col 1024 + 260*s + 65*jj + e,
                # row r -> V_aug[b, vb*512 + jj*128 + r, e]
                blk = v_aug[b, vb * 512:(vb + 1) * 512, :].reshape(4, 128, 65)
                inb[p, :, 1024 + 260 * s:1024 + 260 * (s + 1)] = (
                    blk.transpose(1, 0, 2).reshape(128, 260).view(np.float16)
                )
        in_maps.append({"inb": inb, "trm": trm, "stm": stm})

    nc = _get_nc()
    res = run_bass_kernel_spmd(
        nc, in_maps, list(range(8)),
        trace=TRACE,
        trace_cores=list(range(8)) if TRACE else None,
    )
    LAST_RESULTS = res

    # gather: sum per-(b, qc) segment partials, normalize, transpose back
    acc = np.zeros((B, 8, 65, 512), dtype=np.float64)
    for c in range(8):
        bp, kk = divmod(c, 4)
        o = res.results[c]["out"]  # [8, 65, 512]
        for seg in range(4):
            qc = STREAMS[kk][seg][0]
            for s, b in enumerate((2 * bp, 2 * bp + 1)):
                acc[b, qc] += o[2 * seg + s]
    denom = acc[:, :, 64:65, :]
    denom = np.where(denom == 0.0, 1.0, denom)
    o_t = acc[:, :, 0:64, :] / denom                      # [B, 8, 64, 512]
    out = o_t.transpose(0, 1, 3, 2).reshape(B, T, D)      # [B, T, D]
    out = out * q_mask[:, :, None]
    return out.astype(np.float32)
